# revision 11
# baseline (speedup 1.0000x reference)
"""Trainium2 Bass kernel for nn_Deep_AD (anisotropic-diffusion CNN).

Math per step t (T=3), on x [N,1,512,512]:
  d  = conv2d_same(x, W[t]) + b[t]          # 1 -> 8 channels, 3x3
  f  = exp(-|d| / (1 + d^2)) * d
  x  = x - sum_ch(f) / 8

Distribution: pure data parallel, 2 images/core on 8 cores (batch shard).

Per-core scheme (G approximated by a cubic in |d|, as baseline):
  x_new = [x - (sum_ch d)/8] - (sum_ch f_res)/8,  f_res = d*(G(|d|)-1)

  * The bracketed LINEAR part is computed on the PE: per 32-row window one
    "lin" matmul whose lhsT encodes delta - sum_ch(W)/8 (plus bias row),
    reading the same interleaved rhs as the conv. It accumulates into the
    per-chunk psum, so psum ends up holding x_new directly.
  * conv: per window 2 strip matmuls [103,128]x[103,512] (fp16, banded lhsT
    over a dx-major interleave: rhs partition 34*dx+rr).
  * elementwise f_res: most windows ONE custom DVE op (cubic, reads conv
    psum fp32, writes fp8e4m3 scaled x16). Offloaded windows use a factored
    cubic: a=|d| [ACT], w=(a+P/2)^2 [ACT], v=sigma*a*d [Pool STT],
    h=(w+Q2)*v [Pool STT] with f_res = c2/sigma * h (sign/scale folded into
    the channel-sum weights).
  * channel sum: ONE fp8 DoubleRow matmul per window: the 2 strips are the
    2 k-tiles; selector lhsT routes each strip to its 16 psum rows.
  * drain: ACT copies psum (= x_new) to fp16 canon (last step: fp32 y_sb).
  * DRAM staging: plain padded image V [514,514] fp16; write = 1 DMA from
    canon; loads = 3 DMAs (one per dx) into smega [34*dx+rr, win, col].
"""

import numpy as np
import ml_dtypes

import concourse.bacc as bacc
import concourse.bass as bass
import concourse.tile as tile
from concourse import mybir
from concourse.bass_utils import run_bass_kernel_spmd
from concourse import dve_ops as _dve_ops
from concourse.dve_spec import (
    C0 as _C0,
    C1 as _C1,
    C2 as _C2,
    AluOp as _DAlu,
    Bin as _DBin,
    Spec as _DSpec,
    Src0 as _S0,
)


def _register_dve_op(name, spec, perf_en=None):
    """Register a custom DVE op at runtime, probing the uops sha."""
    import re as _re

    for op in _dve_ops.OPS:
        if op.name == name:
            return op
    probe = _dve_ops.DveOp(name, spec, subdim=False, uops_sha={}, perf_en=perf_en or {})
    _dve_ops.OPS.append(probe)
    _dve_ops._SUB_OPCODE_FOR_NAME[name] = _dve_ops._CUSTOM_DVE_ROW_BASE + len(_dve_ops.OPS) - 1
    shas = {}
    for ver in ("v3", "v4"):
        try:
            probe.compile(ver)
        except ValueError as e:
            m = _re.search(r"\(" + ver + r": ([0-9a-f]+) ", str(e))
            if not m:
                raise
            shas[ver] = m.group(1)
    final = _dve_ops.DveOp(
        name, spec, subdim=False, uops_sha=shas, perf_en=perf_en or {}
    )
    _dve_ops.OPS[-1] = final
    _dve_ops.CUSTOM_DVE_SPECS[name] = spec
    return final


# cubic fit of exp(-|d|/(1+d^2)) (weighted by the empirical |d| distribution
# of this problem + a small uniform floor on [0,2.75] to bound tail error).
QC0, QC1, QC2 = -0.91730678, 0.61782336, -0.11857027
FSC = 16.0                       # DVE-path f_res output scale (lo = -1/128)
# factored form: f_res = c2*(a^2 + P*a + Q)*(a*d)
P_F = QC1 / QC2
Q_F = QC0 / QC2
LO_CHAIN = 0.015625              # +2^-6, e4m3-exact; = |c2|/(8*sigma)
SIGMA = abs(QC2) / (8.0 * LO_CHAIN)
Q2_F = Q_F - (P_F / 2.0) ** 2    # for w = (a + P/2)^2 built via ACT Square


def _ref_adcubr(in0, in1, s0, s1, imm2):
    d = in0.astype(np.float32)
    a = np.abs(d)
    return ((imm2 * a + s1) * a + s0) * a * d


_a = _DBin(_DAlu.ABSOLUTE_VALUE, _S0, _S0)
_t5 = (((_C2 * _a) + _C1) * _a + _C0) * _a
AD_CUBR = _register_dve_op(
    "AD_CUBR",
    _DSpec(body=_t5 * _S0, reference=_ref_adcubr),
)

# problem constants (hardcoded; kernel.py must be self-contained)
T, KCH, H, W_IMG = 3, 8, 512, 512
N_IMG, N_CORES, IPC = 16, 8, 2
WIN, NWIN, NCHUNK = 32, 16, 4      # 32-row windows; 128-row chunks
INR = 34                           # input rows per window (SAME pad)
BIAS_P = 3 * INR                   # partition 102: constant-ones row (bias)
FD = 1024                          # 2 strips per window flush
VP = H + 2                         # padded image dim (514)
DT = mybir.dt.float32
BF = mybir.dt.float16
F8 = mybir.dt.float8e4
NP8 = ml_dtypes.float8_e4m3
# windows whose elementwise runs on the ACT+Pool chain (per image-step)
CHAIN_WS = frozenset({1, 4, 7, 10, 13})


def _host_lhst(W, b):
    """Host-built stationary tensors.

    lc  [T*2,128,128] f16 : conv lhsT, strip parity sp; row 34*dx+(16sp+g+ky)
                            -> out col 8g+ch; row 102 = bias.
    ln  [T,128,32]   f16  : lin lhsT; encodes delta - sum_ch(W)/8 and -sum(b)/8.
    lo  [8,128,2,128] f8  : ones lhsT variants [2*(w%4)+is_chain]; k-tile kt
                            routes strip 2(w%4)+kt to psum rows 16j+g.
    """
    W = np.asarray(W, np.float32)
    b = np.asarray(b, np.float32)
    lc = np.zeros((T, 2, 128, 128), np.float32)
    for t in range(T):
        for sp in range(2):
            for g in range(16):
                for ch in range(KCH):
                    m = 8 * g + ch
                    for ky in range(3):
                        rp = 16 * sp + g + ky
                        for kx in range(3):
                            lc[t, sp, 34 * kx + rp, m] = W[t, ch, 0, ky, kx]
                    lc[t, sp, BIAS_P, m] = b[t, ch]
    ln = np.zeros((T, 128, 32), np.float32)
    Wsum = W.sum(axis=1)[:, 0]  # [T,3,3]
    for t in range(T):
        for r in range(32):
            for ky in range(3):
                for kx in range(3):
                    v = -Wsum[t, ky, kx] / KCH
                    if ky == 1 and kx == 1:
                        v += 1.0
                    ln[t, 34 * kx + (r + ky), r] += v
            ln[t, BIAS_P, r] = -b[t].sum() / KCH
    lo = np.zeros((4, 2, 128, 2, 128), np.float32)
    for wm in range(4):
        for kt in range(2):
            j = 2 * wm + kt
            for g in range(16):
                for ch in range(KCH):
                    lo[wm, 0, 8 * g + ch, kt, 16 * j + g] = -1.0 / (KCH * FSC)
                    lo[wm, 1, 8 * g + ch, kt, 16 * j + g] = LO_CHAIN
    return (
        lc.reshape(T * 2, 128, 128).astype(np.float16),
        ln.astype(np.float16),
        lo.reshape(8, 128, 2, 128).astype(NP8),
    )


def build_nc():
    nc = bacc.Bacc(None)
    x_d = nc.declare_dram_parameter("x", [IPC, H, W_IMG], DT, isOutput=False)
    lc_d = nc.declare_dram_parameter("lc", [T * 2, 128, 128], BF, isOutput=False)
    ln_d = nc.declare_dram_parameter("ln", [T, 128, 32], BF, isOutput=False)
    lo_d = nc.declare_dram_parameter("lo", [8, 128, 2, 128], F8, isOutput=False)
    cstb_d = nc.declare_dram_parameter("cstb", [2, VP], BF, isOutput=False)
    y_d = nc.declare_dram_parameter("y", [IPC, H, W_IMG], DT, isOutput=True)

    DR = mybir.MatmulPerfMode.DoubleRow
    A = mybir.ActivationFunctionType
    ALU = mybir.AluOpType

    with tile.TileContext(nc) as tc:
        from contextlib import ExitStack

        ctx = ExitStack()
        with ctx:
            singles = ctx.enter_context(tc.tile_pool(name="singles", bufs=1))
            p_conv = ctx.enter_context(
                tc.tile_pool(name="p_conv", bufs=2, space="PSUM")
            )
            p_ones = ctx.enter_context(
                tc.tile_pool(name="p_ones", bufs=2, space="PSUM")
            )
            ew_f = ctx.enter_context(tc.tile_pool(name="ew_f", bufs=10))
            ch_a = ctx.enter_context(tc.tile_pool(name="ch_a", bufs=3))
            ch_w = ctx.enter_context(tc.tile_pool(name="ch_w", bufs=3))
            ch_v = ctx.enter_context(tc.tile_pool(name="ch_v", bufs=3))

            # per-partition scalar constant for the chain's Square bias
            half_p = singles.tile([128, 1], DT, name="half_p")
            nc.gpsimd.memset(half_p[:, :], P_F / 2.0)

            # stationary operands to SBUF
            lc_sb = singles.tile([128, T * 2, 128], BF)
            nc.sync.dma_start(out=lc_sb, in_=lc_d.rearrange("v k m -> k v m"))
            ln_sb = singles.tile([128, T, 32], BF)
            nc.sync.dma_start(out=ln_sb, in_=ln_d.rearrange("v k m -> k v m"))
            lo_sb = singles.tile([128, 8, 2, 128], F8)
            nc.sync.dma_start(
                out=lo_sb, in_=lo_d.rearrange("v k t m -> k v t m")
            )

            # canonical x staging [128, chunk, 514] fp16, one per image; only
            # cols 1..512 are written by drains -> zero the padding cols once.
            canon = [
                singles.tile([128, NCHUNK, VP], BF, name=f"canon_{i}")
                for i in range(IPC)
            ]
            zc = cstb_d[1:2, 0:1]
            for i in range(IPC):
                zcol = bass.AP(
                    tensor=zc.tensor,
                    offset=zc.offset,
                    ap=[[0, 128], [0, NCHUNK], [1, 1]],
                )
                nc.sync.dma_start(out=canon[i][:, :, 0:1], in_=zcol)
                nc.sync.dma_start(out=canon[i][:, :, 513:514], in_=zcol)

            # final-step output staging fp32
            y_sb = [
                singles.tile([128, NCHUNK, 512], DT, name=f"ysb_{i}")
                for i in range(IPC)
            ]

            # interleaved conv rhs [34*dx+rr, win, col] fp16, double-buffered
            smega = [
                [
                    singles.tile([128, NWIN, 512], BF, name=f"sm_{i}_{p}")
                    for p in range(2)
                ]
                for i in range(IPC)
            ]
            ones_row = cstb_d[0:1, 0:1]
            for i in range(IPC):
                for p in range(2):
                    bt = smega[i][p][BIAS_P : BIAS_P + 1, :, :]
                    ones_src = bass.AP(
                        tensor=ones_row.tensor,
                        offset=ones_row.offset,
                        ap=[[0, 1], [0, NWIN], [1, 512]],
                    )
                    nc.sync.dma_start(out=bt, in_=ones_src)

            # plain padded image in DRAM, [VP, VP] fp16, per (img, parity)
            V = [nc.dram_tensor(f"V_{i}", [2, VP, VP], BF) for i in range(IPC)]
            zrow = cstb_d[1:2, 0:1]
            for i in range(IPC):
                for p in range(2):
                    vb = V[i][p]
                    # zero rows 0 and 513 (full width)
                    vdst = bass.AP(
                        tensor=vb.tensor,
                        offset=vb.offset,
                        ap=[[(VP - 1) * VP, 2], [1, VP]],
                    )
                    vsrc = bass.AP(
                        tensor=zrow.tensor, offset=zrow.offset, ap=[[0, 2], [0, VP]]
                    )
                    nc.sync.dma_start(out=vdst, in_=vsrc)
                    # zero cols 0 and 513 (rows 1..512); tiny init-only DMAs
                    with nc.allow_non_contiguous_dma(reason="512x2B pad cols, init only"):
                        for col in (0, VP - 1):
                            vdst2 = bass.AP(
                                tensor=vb.tensor,
                                offset=vb.offset + VP + col,
                                ap=[[VP, H], [1, 1]],
                            )
                            vsrc2 = bass.AP(
                                tensor=zrow.tensor,
                                offset=zrow.offset,
                                ap=[[0, H], [1, 1]],
                            )
                            nc.sync.dma_start(out=vdst2, in_=vsrc2)

            # initial x -> V[i][0] interior (fp32 -> fp16 cast via gpsimd DMA)
            for i in range(IPC):
                xi = x_d[i]
                xsrc = bass.AP(
                    tensor=xi.tensor,
                    offset=xi.offset,
                    ap=[[W_IMG, H], [1, W_IMG]],
                )
                vb = V[i][0]
                vdst = bass.AP(
                    tensor=vb.tensor,
                    offset=vb.offset + VP + 1,
                    ap=[[VP, H], [1, W_IMG]],
                )
                nc.gpsimd.dma_start(out=vdst, in_=xsrc)

            # ---- per-step events ------------------------------------------
            def emit_load(i, t, dx):
                vb = V[i][t % 2]
                src = bass.AP(
                    tensor=vb.tensor,
                    offset=vb.offset + dx,
                    ap=[[VP, INR], [WIN * VP, NWIN], [1, 512]],
                )
                sm = smega[i][t % 2]
                nc.sync.dma_start(out=sm[34 * dx : 34 * dx + INR, :, :], in_=src)

            # per-(img) open psum state
            ones_tiles = [None] * IPC

            def emit_phase1(i, t, w):
                """conv strips for window w."""
                sm = smega[i][t % 2]
                cps = p_conv.tile([128, FD], DT)
                for sp in range(2):
                    nc.tensor.matmul(
                        cps[:, sp * 512 : (sp + 1) * 512],
                        lc_sb[0:103, t * 2 + sp, :],
                        sm[0:103, w, :],
                        start=True,
                        stop=True,
                    )
                return cps

            def emit_lin(i, t, w):
                """lin matmul (x - sum_ch(d)/8 contribution) for window w."""
                sm = smega[i][t % 2]
                j = w % 4
                nc.tensor.matmul(
                    ones_tiles[i][32 * j : 32 * j + 32, :],
                    ln_sb[0:103, t, :],
                    sm[0:103, w, :],
                    start=False,
                    stop=False,
                    skip_group_check=True,
                    tile_position=(0, 32 * j),
                )

            def emit_phase2(i, t, w, cps):
                """elementwise f_res + ones matmul (+ drain on w%4==3)."""
                f8t = ew_f.tile([128, 2, 512], F8)
                is_chain = w in CHAIN_WS
                if is_chain:
                    a_t = ch_a.tile([128, FD], BF)
                    nc.scalar.activation(a_t[:, :], cps[:, 0:FD], A.Abs)
                    w_t = ch_w.tile([128, FD], BF)
                    nc.scalar.activation(
                        w_t[:, :], a_t[:, :], A.Square, bias=half_p[:, 0:1]
                    )
                    v_t = ch_v.tile([128, FD], BF)
                    nc.gpsimd.scalar_tensor_tensor(
                        v_t[:, :], a_t[:, :], SIGMA, cps[:, 0:FD],
                        ALU.mult, ALU.mult,
                    )
                    nc.gpsimd.scalar_tensor_tensor(
                        f8t[:, :, :], w_t[:, :], Q2_F, v_t[:, :],
                        ALU.add, ALU.mult,
                    )
                else:
                    nc.vector._custom_dve(
                        AD_CUBR,
                        out=f8t[:, :, :],
                        in0=cps[:, 0:FD],
                        s0=FSC * QC0, s1=FSC * QC1, imm2=FSC * QC2,
                    )
                j = w % 4
                if j == 0:
                    ones_tiles[i] = p_ones.tile([128, 512], DT, name="ones_ps")
                else:
                    emit_lin(i, t, w)
                nc.tensor.matmul(
                    ones_tiles[i][:, :],
                    lo_sb[:, 2 * j + (1 if is_chain else 0), :, :],
                    f8t[:, :, :],
                    start=(j == 0),
                    stop=(j == 3),
                    perf_mode=DR,
                    skip_group_check=True,
                )
                if j == 0:
                    emit_lin(i, t, w)
                if j == 3:
                    c = w // 4
                    if t < T - 1:
                        nc.scalar.copy(
                            canon[i][:, c, 1:513], ones_tiles[i][:, :]
                        )
                    else:
                        nc.scalar.copy(y_sb[i][:, c, :], ones_tiles[i][:, :])

            def emit_tail(i, t):
                if t < T - 1:
                    vb = V[i][(t + 1) % 2]
                    vdst = bass.AP(
                        tensor=vb.tensor,
                        offset=vb.offset + VP,
                        ap=[[VP, 128], [128 * VP, NCHUNK], [1, VP]],
                    )
                    nc.sync.dma_start(out=vdst, in_=canon[i][:, :, 0:VP])
                else:
                    yi = y_d[i]
                    ydst = bass.AP(
                        tensor=yi.tensor,
                        offset=yi.offset,
                        ap=[[512, 128], [512 * 128, NCHUNK], [1, 512]],
                    )
                    nc.sync.dma_start(out=ydst, in_=y_sb[i][:, :, :])

            # ---- software pipeline: 2 images, image B offset by half a step
            def stream(i):
                ev = []
                for t in range(T):
                    for dx in range(3):
                        ev.append(("ld", i, t, dx))
                    for w in range(NWIN):
                        ev.append(("p1", i, t, w))
                        ev.append(("p2", i, t, w))
                    ev.append(("tl", i, t))
                return ev

            def run(ev_list):
                cps_live = {}
                for ev in ev_list:
                    kind = ev[0]
                    if kind == "ld":
                        emit_load(ev[1], ev[2], ev[3])
                    elif kind == "p1":
                        cps_live[(ev[1], ev[3])] = emit_phase1(ev[1], ev[2], ev[3])
                    elif kind == "p2":
                        emit_phase2(ev[1], ev[2], ev[3], cps_live.pop((ev[1], ev[3])))
                    else:
                        emit_tail(ev[1], ev[2])

            sa, sb = stream(0), stream(1)
            # interleave event-by-event with B trailing by half a step's
            # events so step-boundary DMA latency of one image hides under
            # the other's compute.
            STAG = 3 + NWIN  # B starts after A's loads + half its windows... tuned
            merged = []
            pa = pb = 0
            while pa < len(sa) or pb < len(sb):
                if pa < len(sa) and (pa < STAG or pa <= pb + STAG):
                    merged.append(sa[pa])
                    pa += 1
                if pb < len(sb) and pa >= STAG:
                    merged.append(sb[pb])
                    pb += 1
            run(merged)

    nc.compile()
    return nc


_NC_CACHE = None


def _get_nc():
    global _NC_CACHE
    if _NC_CACHE is None:
        _NC_CACHE = build_nc()
    return _NC_CACHE


def make_in_maps(x, W, b):
    x = np.asarray(x, np.float32).reshape(N_IMG, H, W_IMG)
    lc, ln, lo = _host_lhst(W, b)
    cstb = np.stack(
        [np.ones(VP, np.float32), np.zeros(VP, np.float32)]
    ).astype(np.float16)
    return [
        {
            "x": np.ascontiguousarray(x[IPC * c : IPC * (c + 1)]),
            "lc": lc,
            "ln": ln,
            "lo": lo,
            "cstb": cstb,
        }
        for c in range(N_CORES)
    ]


def kernel(x, W, b):
    nc = _get_nc()
    in_maps = make_in_maps(x, W, b)
    res = run_bass_kernel_spmd(nc, in_maps, list(range(N_CORES))).results
    out = np.stack([res[c]["y"] for c in range(N_CORES)])  # [8, 2, 512, 512]
    return out.reshape(N_IMG, 1, H, W_IMG)


if __name__ == "__main__":
    # CoreSim self-test on one core's shard
    from concourse import bass_interp

    rng = np.random.default_rng(0)
    x = rng.standard_normal((IPC, H, W_IMG)).astype(np.float32)
    W = (rng.standard_normal((T, KCH, 1, 3, 3)) * 0.1).astype(np.float32)
    b = (rng.standard_normal((T, KCH)) * 0.1).astype(np.float32)

    def ref_np(x, W, b):
        from scipy.signal import correlate2d

        cur = x.copy()
        for t in range(T):
            d = np.stack(
                [
                    np.stack(
                        [
                            correlate2d(cur[n], W[t, k, 0], mode="same")
                            for k in range(KCH)
                        ]
                    )
                    for n in range(IPC)
                ]
            ) + b[t][None, :, None, None]
            f = np.exp(-np.abs(d) / (1.0 + d * d)) * d
            cur = cur - f.sum(axis=1) / KCH
        return cur

    nc = build_nc()
    lc, ln, lo = _host_lhst(W, b)
    sim = bass_interp.CoreSim(nc)
    sim.tensor("x")[:] = x
    sim.tensor("lc")[:] = lc
    sim.tensor("ln")[:] = ln
    sim.tensor("lo")[:] = lo
    cstf = np.stack([np.ones(VP, np.float32), np.zeros(VP, np.float32)])
    sim.tensor("cstb")[:] = cstf.astype(np.float16)
    sim.simulate()
    got = sim.tensor("y")
    want = ref_np(x, W, b)
    num = np.linalg.norm((got - want).ravel())
    den = np.linalg.norm(want.ravel())
    print("L2 rel:", num / den)
    print("abs max:", np.abs(got - want).max())


# revision 12
# speedup vs baseline: 1.3645x; 1.3645x over previous
"""Trainium2 Bass kernel for nn_Deep_AD (anisotropic-diffusion CNN).

Math per step t (T=3), on x [N,1,512,512]:
  d  = conv2d_same(x, W[t]) + b[t]          # 1 -> 8 channels, 3x3
  f  = exp(-|d| / (1 + d^2)) * d
  x  = x - sum_ch(f) / 8

Distribution: pure data parallel, 2 images/core on 8 cores (batch shard).

Per-core scheme (G approximated by a cubic in |d|, as baseline):
  x_new = [x - (sum_ch d)/8] - (sum_ch f_res)/8,  f_res = d*(G(|d|)-1)

  * The bracketed LINEAR part (minus the bias term) is computed on the PE:
    per 32-row window one "lin" matmul whose lhsT encodes
    delta - sum_ch(W)/8, reading the same interleaved rhs as the conv. It
    accumulates into the per-chunk psum, so psum ends up holding x_new
    (its -sum(b)/8 constant is added at drain time via a per-partition
    bias AP on the ACT Identity copy).
  * conv: per window 2 strip matmuls [102,128]x[102,512] fp16; banded lhsT
    over the baseline rr-major interleave (rhs partition 3*rr+dx). The
    conv bias is NOT in the matmul; it rides into the elementwise ops.
  * elementwise f_res on d+b: most windows ONE custom DVE op (cubic with
    per-partition bias in1, reads conv psum fp32, writes fp8e4m3 scaled
    x16). Offloaded windows: a=|d+b| [ACT Abs, bias AP],
    w2=sigma*(a+P/2)^2 [ACT Square, scale=sqrt(sigma)], v=(d+b)*a
    [Pool STT, scalar=bias AP], h=(w2+sigma*Q2)*v [Pool STT]; then
    f_res = c2/sigma * h with sign/scale folded into the channel-sum lhsT.
  * channel sum: ONE fp8 DoubleRow matmul per window: the 2 strips are the
    2 k-tiles; selector lhsT routes each strip to its 16 psum rows.
  * drain: ACT Identity copies psum (= x_new) +bias2 to fp16 canon (last
    step: fp32 y_sb).
  * DRAM staging V (baseline layout): 3x row-interleaved shifted image,
    V row 3*R+dx = x_pad row R cols dx..dx+511, fp16. write = 3 DMAs from
    canon (col-shifted reads); loads = 2 half DMAs [102, 8, 512].
"""

import numpy as np
import ml_dtypes

import concourse.bacc as bacc
import concourse.bass as bass
import concourse.tile as tile
from concourse import mybir
from concourse.bass_utils import run_bass_kernel_spmd
from concourse import dve_ops as _dve_ops
from concourse.dve_spec import (
    C0 as _C0,
    C1 as _C1,
    C2 as _C2,
    AluOp as _DAlu,
    Bin as _DBin,
    Spec as _DSpec,
    Src0 as _S0,
    Src1 as _S1,
)


def _register_dve_op(name, spec, perf_en=None):
    """Register a custom DVE op at runtime, probing the uops sha."""
    import re as _re

    for op in _dve_ops.OPS:
        if op.name == name:
            return op
    probe = _dve_ops.DveOp(name, spec, subdim=False, uops_sha={}, perf_en=perf_en or {})
    _dve_ops.OPS.append(probe)
    _dve_ops._SUB_OPCODE_FOR_NAME[name] = _dve_ops._CUSTOM_DVE_ROW_BASE + len(_dve_ops.OPS) - 1
    shas = {}
    for ver in ("v3", "v4"):
        try:
            probe.compile(ver)
        except ValueError as e:
            m = _re.search(r"\(" + ver + r": ([0-9a-f]+) ", str(e))
            if not m:
                raise
            shas[ver] = m.group(1)
    final = _dve_ops.DveOp(
        name, spec, subdim=False, uops_sha=shas, perf_en=perf_en or {}
    )
    _dve_ops.OPS[-1] = final
    _dve_ops.CUSTOM_DVE_SPECS[name] = spec
    return final


# cubic fit of exp(-|d|/(1+d^2)) (weighted by the empirical |d| distribution
# of this problem + a small uniform floor on [0,2.75] to bound tail error).
QC0, QC1, QC2 = -0.91730678, 0.61782336, -0.11857027
FSC = 16.0                       # DVE-path f_res output scale (lo = -1/128)
# factored form: f_res = c2*(a^2 + P*a + Q)*(a*(d+b))
P_F = QC1 / QC2
Q_F = QC0 / QC2
LO_CHAIN = 0.015625              # +2^-6, e4m3-exact; = |c2|/(8*sigma)
SIGMA = abs(QC2) / (8.0 * LO_CHAIN)
Q2_F = Q_F - (P_F / 2.0) ** 2    # w2/sigma = (a + P/2)^2 -> +Q2 completes


def _ref_adcubrb(in0, in1, s0, s1, imm2):
    d = in0.astype(np.float32) + in1.astype(np.float32)
    a = np.abs(d)
    return ((imm2 * a + s1) * a + s0) * a * d


_db = _DBin(_DAlu.ADD, _S0, _S1)
_a = _DBin(_DAlu.ABSOLUTE_VALUE, _db, _db)
_t5 = (((_C2 * _a) + _C1) * _a + _C0) * _a
AD_CUBRB = _register_dve_op(
    "AD_CUBRB",
    _DSpec(body=_t5 * _db, reference=_ref_adcubrb),
)

# problem constants (hardcoded; kernel.py must be self-contained)
T, KCH, H, W_IMG = 3, 8, 512, 512
N_IMG, N_CORES, IPC = 16, 8, 2
WIN, NWIN, NCHUNK = 32, 16, 4      # 32-row windows; 128-row chunks
INR = 34                           # input rows per window (SAME pad)
FD = 1024                          # 2 strips per window flush
NVROW = 3 * (H + 2)                # 1542 interleaved V rows per image
DT = mybir.dt.float32
BF = mybir.dt.float16
F8 = mybir.dt.float8e4
NP8 = ml_dtypes.float8_e4m3
# windows whose elementwise runs on the ACT+Pool chain (per image-step)
CHAIN_WS = frozenset({1, 4, 7, 10, 13})


def _host_lhst(W, b):
    """Host-built stationary tensors.

    lc  [T*2,128,128] f16 : conv lhsT, strip parity sp; row 3*(16sp+g+ky)+kx
                            -> out col 8g+ch (rr-major interleave). No bias.
    ln  [T,128,32]   f16  : lin lhsT; encodes delta - sum_ch(W)/8.
    lo  [8,128,2,128] f8  : ones lhsT variants [2*(w%4)+is_chain]; k-tile kt
                            routes strip 2(w%4)+kt to psum rows 16j+g.
    bia [128, 2*T]   f32  : col t = conv bias b[t,ch] per psum partition;
                            col T+t = -sum(b[t])/8 (drain bias).
    """
    W = np.asarray(W, np.float32)
    b = np.asarray(b, np.float32)
    lc = np.zeros((T, 2, 128, 128), np.float32)
    for t in range(T):
        for sp in range(2):
            for g in range(16):
                for ch in range(KCH):
                    m = 8 * g + ch
                    for ky in range(3):
                        rp = 16 * sp + g + ky
                        for kx in range(3):
                            lc[t, sp, 3 * rp + kx, m] = W[t, ch, 0, ky, kx]
    ln = np.zeros((T, 128, 32), np.float32)
    Wsum = W.sum(axis=1)[:, 0]  # [T,3,3]
    for t in range(T):
        for r in range(32):
            for ky in range(3):
                for kx in range(3):
                    v = -Wsum[t, ky, kx] / KCH
                    if ky == 1 and kx == 1:
                        v += 1.0
                    ln[t, 3 * (r + ky) + kx, r] += v
    lo = np.zeros((4, 2, 128, 2, 128), np.float32)
    for wm in range(4):
        for kt in range(2):
            j = 2 * wm + kt
            for g in range(16):
                for ch in range(KCH):
                    lo[wm, 0, 8 * g + ch, kt, 16 * j + g] = -1.0 / (KCH * FSC)
                    lo[wm, 1, 8 * g + ch, kt, 16 * j + g] = LO_CHAIN
    bia = np.zeros((128, 2 * T), np.float32)
    for t in range(T):
        for g in range(16):
            for ch in range(KCH):
                bia[8 * g + ch, t] = b[t, ch]
        bia[:, T + t] = -b[t].sum() / KCH
    return (
        lc.reshape(T * 2, 128, 128).astype(np.float16),
        ln.astype(np.float16),
        lo.reshape(8, 128, 2, 128).astype(NP8),
        bia,
    )


def build_nc():
    nc = bacc.Bacc(None)
    x_d = nc.declare_dram_parameter("x", [IPC, H, W_IMG], DT, isOutput=False)
    lc_d = nc.declare_dram_parameter("lc", [T * 2, 128, 128], BF, isOutput=False)
    ln_d = nc.declare_dram_parameter("ln", [T, 128, 32], BF, isOutput=False)
    lo_d = nc.declare_dram_parameter("lo", [8, 128, 2, 128], F8, isOutput=False)
    bia_d = nc.declare_dram_parameter("bia", [128, 2 * T], DT, isOutput=False)
    cstb_d = nc.declare_dram_parameter("cstb", [2, H + 2], BF, isOutput=False)
    y_d = nc.declare_dram_parameter("y", [IPC, H, W_IMG], DT, isOutput=True)

    DR = mybir.MatmulPerfMode.DoubleRow
    A = mybir.ActivationFunctionType
    ALU = mybir.AluOpType

    with tile.TileContext(nc) as tc:
        from contextlib import ExitStack

        ctx = ExitStack()
        with ctx:
            singles = ctx.enter_context(tc.tile_pool(name="singles", bufs=1))
            p_conv = ctx.enter_context(
                tc.tile_pool(name="p_conv", bufs=2, space="PSUM")
            )
            p_ones = ctx.enter_context(
                tc.tile_pool(name="p_ones", bufs=2, space="PSUM")
            )
            ew_f = ctx.enter_context(tc.tile_pool(name="ew_f", bufs=10))
            ch_a = ctx.enter_context(tc.tile_pool(name="ch_a", bufs=3))
            ch_w = ctx.enter_context(tc.tile_pool(name="ch_w", bufs=3))
            ch_v = ctx.enter_context(tc.tile_pool(name="ch_v", bufs=3))

            # per-partition scalar const for the chain's Square bias
            sqp = singles.tile([128, 1], DT, name="sqp")
            nc.gpsimd.memset(sqp[:, :], (SIGMA ** 0.5) * P_F / 2.0)

            # stationary operands to SBUF
            lc_sb = singles.tile([128, T * 2, 128], BF)
            nc.sync.dma_start(out=lc_sb, in_=lc_d.rearrange("v k m -> k v m"))
            ln_sb = singles.tile([128, T, 32], BF)
            nc.sync.dma_start(out=ln_sb, in_=ln_d.rearrange("v k m -> k v m"))
            lo_sb = singles.tile([128, 8, 2, 128], F8)
            nc.sync.dma_start(
                out=lo_sb, in_=lo_d.rearrange("v k t m -> k v t m")
            )
            bia_sb = singles.tile([128, 2 * T], DT)
            nc.sync.dma_start(out=bia_sb, in_=bia_d[:, :])

            # canonical x staging [128, chunk, 514] fp16, one per image; only
            # cols 1..512 are written by drains -> zero the padding cols once.
            canon = [
                singles.tile([128, NCHUNK, H + 2], BF, name=f"canon_{i}")
                for i in range(IPC)
            ]
            zc = cstb_d[1:2, 0:1]
            for i in range(IPC):
                zcol = bass.AP(
                    tensor=zc.tensor,
                    offset=zc.offset,
                    ap=[[0, 128], [0, NCHUNK], [1, 1]],
                )
                nc.sync.dma_start(out=canon[i][:, :, 0:1], in_=zcol)
                nc.sync.dma_start(out=canon[i][:, :, 513:514], in_=zcol)

            # final-step output staging fp32
            y_sb = [
                singles.tile([128, NCHUNK, 512], DT, name=f"ysb_{i}")
                for i in range(IPC)
            ]

            # interleaved conv rhs [3*rr+dx, win, col] fp16, double-buffered
            smega = [
                [
                    singles.tile([128, NWIN, 512], BF, name=f"sm_{i}_{p}")
                    for p in range(2)
                ]
                for i in range(IPC)
            ]

            # V: DRAM 3x row-interleaved shifted padded image, per (img, par)
            V = [nc.dram_tensor(f"V_{i}", [2, NVROW, 512], BF) for i in range(IPC)]
            zrow = cstb_d[1:2, 0:1]
            for i in range(IPC):
                for p in range(2):
                    vb = V[i][p]
                    vdst = bass.AP(
                        tensor=vb.tensor,
                        offset=vb.offset,
                        ap=[[(NVROW - 3) * 512, 2], [512, 3], [1, 512]],
                    )
                    vsrc = bass.AP(
                        tensor=zrow.tensor,
                        offset=zrow.offset,
                        ap=[[0, 2], [0, 3], [1, 512]],
                    )
                    nc.sync.dma_start(out=vdst, in_=vsrc)

            def write_V(i, par, src_tile, engine):
                """canon (fp16, padded cols) -> V[i][par], 3 DMAs."""
                vb = V[i][par]
                for dx in range(3):
                    vdst = bass.AP(
                        tensor=vb.tensor,
                        offset=vb.offset + (3 + dx) * 512,
                        ap=[[3 * 512, 128], [3 * 128 * 512, NCHUNK], [1, 512]],
                    )
                    engine.dma_start(out=vdst, in_=src_tile[:, :, dx : dx + 512])

            # load input into canon (cast via gpsimd), build V[i][0]
            for i in range(IPC):
                xi = x_d[i]
                xsrc = bass.AP(
                    tensor=xi.tensor,
                    offset=xi.offset,
                    ap=[[512, 128], [512 * 128, NCHUNK], [1, 512]],
                )
                nc.gpsimd.dma_start(out=canon[i][:, :, 1:513], in_=xsrc)
                write_V(i, 0, canon[i], nc.sync)

            # ---- per-step events ------------------------------------------
            def emit_load(i, t, half):
                vb = V[i][t % 2]
                src = bass.AP(
                    tensor=vb.tensor,
                    offset=vb.offset + half * 8 * 96 * 512,
                    ap=[[512, 102], [96 * 512, 8], [1, 512]],
                )
                sm = smega[i][t % 2]
                nc.sync.dma_start(
                    out=sm[0:102, 8 * half : 8 * half + 8, :], in_=src
                )

            # per-(img) open psum state
            ones_tiles = [None] * IPC

            def emit_phase1(i, t, w):
                """conv strips for window w."""
                sm = smega[i][t % 2]
                cps = p_conv.tile([128, FD], DT)
                for sp in range(2):
                    nc.tensor.matmul(
                        cps[:, sp * 512 : (sp + 1) * 512],
                        lc_sb[0:102, t * 2 + sp, :],
                        sm[0:102, w, :],
                        start=True,
                        stop=True,
                    )
                return cps

            def emit_lin(i, t, w):
                """lin matmul (x - sum_ch(d)/8 contribution) for window w."""
                sm = smega[i][t % 2]
                j = w % 4
                nc.tensor.matmul(
                    ones_tiles[i][32 * j : 32 * j + 32, :],
                    ln_sb[0:102, t, :],
                    sm[0:102, w, :],
                    start=False,
                    stop=False,
                    skip_group_check=True,
                    tile_position=(0, 32 * j),
                )

            def emit_phase2(i, t, w, cps):
                """elementwise f_res + ones matmul (+ drain on w%4==3)."""
                f8t = ew_f.tile([128, 2, 512], F8)
                is_chain = w in CHAIN_WS
                bias_ap = bia_sb[:, t : t + 1]
                if is_chain:
                    a_t = ch_a.tile([128, FD], BF)
                    nc.scalar.activation(
                        a_t[:, :], cps[:, 0:FD], A.Abs, bias=bias_ap
                    )
                    w_t = ch_w.tile([128, FD], BF)
                    nc.scalar.activation(
                        w_t[:, :], a_t[:, :], A.Square,
                        bias=sqp[:, 0:1], scale=SIGMA ** 0.5,
                    )
                    v_t = ch_v.tile([128, FD], BF)
                    nc.gpsimd.scalar_tensor_tensor(
                        v_t[:, :], cps[:, 0:FD], bias_ap, a_t[:, :],
                        ALU.add, ALU.mult,
                    )
                    nc.gpsimd.scalar_tensor_tensor(
                        f8t[:, :, :], w_t[:, :], SIGMA * Q2_F, v_t[:, :],
                        ALU.add, ALU.mult,
                    )
                else:
                    nc.vector._custom_dve(
                        AD_CUBRB,
                        out=f8t[:, :, :],
                        in0=cps[:, 0:FD],
                        in1=bias_ap,
                        s0=FSC * QC0, s1=FSC * QC1, imm2=FSC * QC2,
                    )
                j = w % 4
                if j == 0:
                    ones_tiles[i] = p_ones.tile([128, 512], DT, name="ones_ps")
                else:
                    emit_lin(i, t, w)
                nc.tensor.matmul(
                    ones_tiles[i][:, :],
                    lo_sb[:, 2 * j + (1 if is_chain else 0), :, :],
                    f8t[:, :, :],
                    start=(j == 0),
                    stop=(j == 3),
                    perf_mode=DR,
                    skip_group_check=True,
                )
                if j == 0:
                    emit_lin(i, t, w)
                if j == 3:
                    c = w // 4
                    drain_bias = bia_sb[:, T + t : T + t + 1]
                    if t < T - 1:
                        nc.scalar.activation(
                            canon[i][:, c, 1:513], ones_tiles[i][:, :],
                            A.Identity, bias=drain_bias,
                        )
                    else:
                        nc.scalar.activation(
                            y_sb[i][:, c, :], ones_tiles[i][:, :],
                            A.Identity, bias=drain_bias,
                        )

            def emit_tail(i, t):
                if t < T - 1:
                    # spread the 3 write DMAs across queues
                    write_V(i, (t + 1) % 2, canon[i], nc.scalar)
                else:
                    yi = y_d[i]
                    ydst = bass.AP(
                        tensor=yi.tensor,
                        offset=yi.offset,
                        ap=[[512, 128], [512 * 128, NCHUNK], [1, 512]],
                    )
                    nc.sync.dma_start(out=ydst, in_=y_sb[i][:, :, :])

            # ---- software pipeline: 2 images, image B offset by half a step
            def stream(i):
                ev = []
                for t in range(T):
                    for half in range(2):
                        ev.append(("ld", i, t, half))
                    for w in range(NWIN):
                        ev.append(("p1", i, t, w))
                        ev.append(("p2", i, t, w))
                    ev.append(("tl", i, t))
                return ev

            def run(ev_list):
                cps_live = {}
                for ev in ev_list:
                    kind = ev[0]
                    if kind == "ld":
                        emit_load(ev[1], ev[2], ev[3])
                    elif kind == "p1":
                        cps_live[(ev[1], ev[3])] = emit_phase1(ev[1], ev[2], ev[3])
                    elif kind == "p2":
                        emit_phase2(ev[1], ev[2], ev[3], cps_live.pop((ev[1], ev[3])))
                    else:
                        emit_tail(ev[1], ev[2])

            sa, sb = stream(0), stream(1)
            STAG = 2 + NWIN  # B trails A by half a step's events
            merged = []
            pa = pb = 0
            while pa < len(sa) or pb < len(sb):
                if pa < len(sa) and (pa < STAG or pa <= pb + STAG):
                    merged.append(sa[pa])
                    pa += 1
                if pb < len(sb) and pa >= STAG:
                    merged.append(sb[pb])
                    pb += 1
            run(merged)

    nc.compile()
    return nc


_NC_CACHE = None


def _get_nc():
    global _NC_CACHE
    if _NC_CACHE is None:
        _NC_CACHE = build_nc()
    return _NC_CACHE


def make_in_maps(x, W, b):
    x = np.asarray(x, np.float32).reshape(N_IMG, H, W_IMG)
    lc, ln, lo, bia = _host_lhst(W, b)
    cstb = np.stack(
        [np.ones(H + 2, np.float32), np.zeros(H + 2, np.float32)]
    ).astype(np.float16)
    return [
        {
            "x": np.ascontiguousarray(x[IPC * c : IPC * (c + 1)]),
            "lc": lc,
            "ln": ln,
            "lo": lo,
            "bia": bia,
            "cstb": cstb,
        }
        for c in range(N_CORES)
    ]


def kernel(x, W, b):
    nc = _get_nc()
    in_maps = make_in_maps(x, W, b)
    res = run_bass_kernel_spmd(nc, in_maps, list(range(N_CORES))).results
    out = np.stack([res[c]["y"] for c in range(N_CORES)])  # [8, 2, 512, 512]
    return out.reshape(N_IMG, 1, H, W_IMG)


if __name__ == "__main__":
    # CoreSim self-test on one core's shard
    from concourse import bass_interp

    rng = np.random.default_rng(0)
    x = rng.standard_normal((IPC, H, W_IMG)).astype(np.float32)
    W = (rng.standard_normal((T, KCH, 1, 3, 3)) * 0.1).astype(np.float32)
    b = (rng.standard_normal((T, KCH)) * 0.1).astype(np.float32)

    def ref_np(x, W, b):
        from scipy.signal import correlate2d

        cur = x.copy()
        for t in range(T):
            d = np.stack(
                [
                    np.stack(
                        [
                            correlate2d(cur[n], W[t, k, 0], mode="same")
                            for k in range(KCH)
                        ]
                    )
                    for n in range(IPC)
                ]
            ) + b[t][None, :, None, None]
            f = np.exp(-np.abs(d) / (1.0 + d * d)) * d
            cur = cur - f.sum(axis=1) / KCH
        return cur

    nc = build_nc()
    lc, ln, lo, bia = _host_lhst(W, b)
    sim = bass_interp.CoreSim(nc)
    sim.tensor("x")[:] = x
    sim.tensor("lc")[:] = lc
    sim.tensor("ln")[:] = ln
    sim.tensor("lo")[:] = lo
    sim.tensor("bia")[:] = bia
    cstf = np.stack([np.ones(H + 2, np.float32), np.zeros(H + 2, np.float32)])
    sim.tensor("cstb")[:] = cstf.astype(np.float16)
    sim.simulate()
    got = sim.tensor("y")
    want = ref_np(x, W, b)
    num = np.linalg.norm((got - want).ravel())
    den = np.linalg.norm(want.ravel())
    print("L2 rel:", num / den)
    print("abs max:", np.abs(got - want).max())


# revision 16
# speedup vs baseline: 2.0231x; 1.4826x over previous
"""Trainium2 Bass kernel for nn_Deep_AD (anisotropic-diffusion CNN).

Math per step t (T=3), on x [N,1,512,512]:
  d  = conv2d_same(x, W[t]) + b[t]          # 1 -> 8 channels, 3x3
  f  = exp(-|d| / (1 + d^2)) * d
  x  = x - sum_ch(f) / 8

Distribution: pure data parallel, 2 images/core on 8 cores (batch shard).

Per-core scheme (G approximated by a cubic in |d|, as baseline):
  x_new = [x - (sum_ch d)/8] - (sum_ch f_res)/8,  f_res = d*(G(|d|)-1)

  * The bracketed LINEAR part (minus the bias term) is computed on the PE:
    per 32-row window one "lin" matmul whose lhsT encodes
    delta - sum_ch(W)/8, reading the same interleaved rhs as the conv. It
    accumulates into the per-chunk psum, so psum ends up holding x_new
    (its -sum(b)/8 constant is added at drain time via a per-partition
    bias AP on the ACT Identity copy).
  * conv: per window 2 strip matmuls [102,128]x[102,512] fp16; banded lhsT
    over the baseline rr-major interleave (rhs partition 3*rr+dx). The
    conv bias is NOT in the matmul; it rides into the elementwise ops.
  * elementwise f_res on d+b: most windows ONE custom DVE op (cubic with
    per-partition bias in1, reads conv psum fp32, writes fp8e4m3 scaled
    x16). Offloaded windows: a=|d+b| [ACT Abs, bias AP],
    w2=sigma*(a+P/2)^2 [ACT Square, scale=sqrt(sigma)], v=(d+b)*a
    [Pool STT, scalar=bias AP], h=(w2+sigma*Q2)*v [Pool STT]; then
    f_res = c2/sigma * h with sign/scale folded into the channel-sum lhsT.
  * channel sum: ONE fp8 DoubleRow matmul per window: the 2 strips are the
    2 k-tiles; selector lhsT routes each strip to its 16 psum rows.
  * drain: ACT Identity copies psum (= x_new) +bias2 to fp16 canon (last
    step: fp32 y_sb).
  * DRAM staging V (baseline layout): 3x row-interleaved shifted image,
    V row 3*R+dx = x_pad row R cols dx..dx+511, fp16. write = 3 DMAs from
    canon (col-shifted reads); loads = 2 half DMAs [102, 8, 512].
"""

import numpy as np
import ml_dtypes

import concourse.bacc as bacc
import concourse.bass as bass
import concourse.tile as tile
from concourse import mybir
from concourse.bass_utils import run_bass_kernel_spmd
from concourse import dve_ops as _dve_ops
from concourse.dve_spec import (
    C0 as _C0,
    C1 as _C1,
    C2 as _C2,
    AluOp as _DAlu,
    Bin as _DBin,
    Spec as _DSpec,
    Src0 as _S0,
    Src1 as _S1,
)


def _register_dve_op(name, spec, perf_en=None):
    """Register a custom DVE op at runtime, probing the uops sha."""
    import re as _re

    for op in _dve_ops.OPS:
        if op.name == name:
            return op
    probe = _dve_ops.DveOp(name, spec, subdim=False, uops_sha={}, perf_en=perf_en or {})
    _dve_ops.OPS.append(probe)
    _dve_ops._SUB_OPCODE_FOR_NAME[name] = _dve_ops._CUSTOM_DVE_ROW_BASE + len(_dve_ops.OPS) - 1
    shas = {}
    for ver in ("v3", "v4"):
        try:
            probe.compile(ver)
        except ValueError as e:
            m = _re.search(r"\(" + ver + r": ([0-9a-f]+) ", str(e))
            if not m:
                raise
            shas[ver] = m.group(1)
    final = _dve_ops.DveOp(
        name, spec, subdim=False, uops_sha=shas, perf_en=perf_en or {}
    )
    _dve_ops.OPS[-1] = final
    _dve_ops.CUSTOM_DVE_SPECS[name] = spec
    return final


# cubic fit of exp(-|d|/(1+d^2)) (weighted by the empirical |d| distribution
# of this problem + a small uniform floor on [0,2.75] to bound tail error).
QC0, QC1, QC2 = -0.91730678, 0.61782336, -0.11857027
FSC = 16.0                       # DVE-path f_res output scale (lo = -1/128)
# factored form: f_res = c2*(a^2 + P*a + Q)*(a*(d+b))
P_F = QC1 / QC2
Q_F = QC0 / QC2
LO_CHAIN = 0.015625              # +2^-6, e4m3-exact; = |c2|/(8*sigma)
SIGMA = abs(QC2) / (8.0 * LO_CHAIN)
Q2_F = Q_F - (P_F / 2.0) ** 2    # w2/sigma = (a + P/2)^2 -> +Q2 completes


def _ref_adcubrb(in0, in1, s0, s1, imm2):
    d = in0.astype(np.float32) + in1.astype(np.float32)
    a = np.abs(d)
    return ((imm2 * a + s1) * a + s0) * a * d


_db = _DBin(_DAlu.ADD, _S0, _S1)
_a = _DBin(_DAlu.ABSOLUTE_VALUE, _db, _db)
_t5 = (((_C2 * _a) + _C1) * _a + _C0) * _a
AD_CUBRB = _register_dve_op(
    "AD_CUBRB",
    _DSpec(body=_t5 * _db, reference=_ref_adcubrb),
)

# problem constants (hardcoded; kernel.py must be self-contained)
T, KCH, H, W_IMG = 3, 8, 512, 512
N_IMG, N_CORES, IPC = 16, 8, 2
WIN, NWIN, NCHUNK = 32, 16, 4      # 32-row windows; 128-row chunks
INR = 34                           # input rows per window (SAME pad)
FD = 1024                          # 2 strips per window flush
NVROW = 3 * (H + 2)                # 1542 interleaved V rows per image
DT = mybir.dt.float32
BF = mybir.dt.float16
F8 = mybir.dt.float8e4
NP8 = ml_dtypes.float8_e4m3
# windows whose elementwise runs on the ACT+Pool chain (per image-step)
CHAIN_WS = frozenset({1, 4, 7, 10, 13})


def _host_lhst(W, b):
    """Host-built stationary tensors.

    lc  [T*2,128,128] f16 : conv lhsT, strip parity sp; row 3*(16sp+g+ky)+kx
                            -> out col 8g+ch (rr-major interleave). No bias.
    ln  [T,128,32]   f16  : lin lhsT; encodes delta - sum_ch(W)/8.
    lo  [8,128,2,128] f8  : ones lhsT variants [2*(w%4)+is_chain]; k-tile kt
                            routes strip 2(w%4)+kt to psum rows 16j+g.
    bia [128, 2*T]   f32  : col t = conv bias b[t,ch] per psum partition;
                            col T+t = -sum(b[t])/8 (drain bias).
    """
    W = np.asarray(W, np.float32)
    b = np.asarray(b, np.float32)
    lc = np.zeros((T, 2, 128, 128), np.float32)
    for t in range(T):
        for sp in range(2):
            for g in range(16):
                for ch in range(KCH):
                    m = 8 * g + ch
                    for ky in range(3):
                        rp = 16 * sp + g + ky
                        for kx in range(3):
                            lc[t, sp, 3 * rp + kx, m] = W[t, ch, 0, ky, kx]
    ln = np.zeros((T, 128, 32), np.float32)
    Wsum = W.sum(axis=1)[:, 0]  # [T,3,3]
    for t in range(T):
        for r in range(32):
            for ky in range(3):
                for kx in range(3):
                    v = -Wsum[t, ky, kx] / KCH
                    if ky == 1 and kx == 1:
                        v += 1.0
                    ln[t, 3 * (r + ky) + kx, r] += v
    lo = np.zeros((4, 2, 128, 2, 128), np.float32)
    for wm in range(4):
        for kt in range(2):
            j = 2 * wm + kt
            for g in range(16):
                for ch in range(KCH):
                    lo[wm, 0, 8 * g + ch, kt, 16 * j + g] = -1.0 / (KCH * FSC)
                    lo[wm, 1, 8 * g + ch, kt, 16 * j + g] = LO_CHAIN
    bia = np.zeros((128, 2 * T), np.float32)
    for t in range(T):
        for g in range(16):
            for ch in range(KCH):
                bia[8 * g + ch, t] = b[t, ch]
        bia[:, T + t] = -b[t].sum() / KCH
    return (
        lc.reshape(T * 2, 128, 128).astype(np.float16),
        ln.astype(np.float16),
        lo.reshape(8, 128, 2, 128).astype(NP8),
        bia,
    )


def build_nc():
    nc = bacc.Bacc(None)
    x_d = nc.declare_dram_parameter("x", [IPC, H, W_IMG], DT, isOutput=False)
    lc_d = nc.declare_dram_parameter("lc", [T * 2, 128, 128], BF, isOutput=False)
    ln_d = nc.declare_dram_parameter("ln", [T, 128, 32], BF, isOutput=False)
    lo_d = nc.declare_dram_parameter("lo", [8, 128, 2, 128], F8, isOutput=False)
    bia_d = nc.declare_dram_parameter("bia", [128, 2 * T], DT, isOutput=False)
    cstb_d = nc.declare_dram_parameter("cstb", [2, H + 2], BF, isOutput=False)
    y_d = nc.declare_dram_parameter("y", [IPC, H, W_IMG], DT, isOutput=True)

    DR = mybir.MatmulPerfMode.DoubleRow
    A = mybir.ActivationFunctionType
    ALU = mybir.AluOpType

    with tile.TileContext(nc) as tc:
        from contextlib import ExitStack

        ctx = ExitStack()
        with ctx:
            singles = ctx.enter_context(tc.tile_pool(name="singles", bufs=1))
            p_conv = ctx.enter_context(
                tc.tile_pool(name="p_conv", bufs=3, space="PSUM")
            )
            p_ones = ctx.enter_context(
                tc.tile_pool(name="p_ones", bufs=2, space="PSUM")
            )
            ew_f = ctx.enter_context(tc.tile_pool(name="ew_f", bufs=10))
            ch_a = ctx.enter_context(tc.tile_pool(name="ch_a", bufs=3))
            ch_w = ctx.enter_context(tc.tile_pool(name="ch_w", bufs=3))
            ch_v = ctx.enter_context(tc.tile_pool(name="ch_v", bufs=3))

            # per-partition scalar const for the chain's Square bias
            sqp = singles.tile([128, 1], DT, name="sqp")
            nc.gpsimd.memset(sqp[:, :], (SIGMA ** 0.5) * P_F / 2.0)

            # stationary operands to SBUF
            lc_sb = singles.tile([128, T * 2, 128], BF)
            nc.sync.dma_start(out=lc_sb, in_=lc_d.rearrange("v k m -> k v m"))
            ln_sb = singles.tile([128, T, 32], BF)
            nc.sync.dma_start(out=ln_sb, in_=ln_d.rearrange("v k m -> k v m"))
            lo_sb = singles.tile([128, 8, 2, 128], F8)
            nc.sync.dma_start(
                out=lo_sb, in_=lo_d.rearrange("v k t m -> k v t m")
            )
            bia_sb = singles.tile([128, 2 * T], DT)
            nc.sync.dma_start(out=bia_sb, in_=bia_d[:, :])

            # canonical x staging [128, chunk, 514] fp16, one per image; only
            # cols 1..512 are written by drains -> zero the padding cols once.
            canon = [
                singles.tile([128, NCHUNK, H + 2], BF, name=f"canon_{i}")
                for i in range(IPC)
            ]
            zc = cstb_d[1:2, 0:1]
            for i in range(IPC):
                zcol = bass.AP(
                    tensor=zc.tensor,
                    offset=zc.offset,
                    ap=[[0, 128], [0, NCHUNK], [1, 1]],
                )
                nc.sync.dma_start(out=canon[i][:, :, 0:1], in_=zcol)
                nc.sync.dma_start(out=canon[i][:, :, 513:514], in_=zcol)

            # final-step output staging fp32
            y_sb = [
                singles.tile([128, NCHUNK, 512], DT, name=f"ysb_{i}")
                for i in range(IPC)
            ]

            # interleaved conv rhs [3*rr+dx, win, col] fp16, double-buffered
            smega = [
                [
                    singles.tile([128, NWIN, 512], BF, name=f"sm_{i}_{p}")
                    for p in range(2)
                ]
                for i in range(IPC)
            ]

            # V: DRAM 3x row-interleaved shifted padded image, per (img, par)
            V = [nc.dram_tensor(f"V_{i}", [2, NVROW, 512], BF) for i in range(IPC)]
            zrow = cstb_d[1:2, 0:1]
            for i in range(IPC):
                for p in range(2):
                    vb = V[i][p]
                    vdst = bass.AP(
                        tensor=vb.tensor,
                        offset=vb.offset,
                        ap=[[(NVROW - 3) * 512, 2], [512, 3], [1, 512]],
                    )
                    vsrc = bass.AP(
                        tensor=zrow.tensor,
                        offset=zrow.offset,
                        ap=[[0, 2], [0, 3], [1, 512]],
                    )
                    nc.sync.dma_start(out=vdst, in_=vsrc)

            def write_V(i, par, src_tile, engines, c0=0, nch=NCHUNK):
                """canon chunks [c0, c0+nch) -> V[i][par], 3 DMAs (one per dx)
                spread across the given engine queues."""
                vb = V[i][par]
                for dx in range(3):
                    vdst = bass.AP(
                        tensor=vb.tensor,
                        offset=vb.offset
                        + (3 * (128 * c0 + 1) + dx) * 512,
                        ap=[[3 * 512, 128], [3 * 128 * 512, nch], [1, 512]],
                    )
                    engines[dx % len(engines)].dma_start(
                        out=vdst, in_=src_tile[:, c0 : c0 + nch, dx : dx + 512]
                    )

            # load input into canon (cast via gpsimd), build V[i][0]
            for i in range(IPC):
                xi = x_d[i]
                xsrc = bass.AP(
                    tensor=xi.tensor,
                    offset=xi.offset,
                    ap=[[512, 128], [512 * 128, NCHUNK], [1, 512]],
                )
                nc.gpsimd.dma_start(out=canon[i][:, :, 1:513], in_=xsrc)
                write_V(i, 0, canon[i], [nc.sync, nc.scalar, nc.gpsimd])

            # ---- per-step events ------------------------------------------
            def emit_load(i, t, q, engine):
                """quarter-load: windows 4q..4q+3 of step t into smega."""
                vb = V[i][t % 2]
                src = bass.AP(
                    tensor=vb.tensor,
                    offset=vb.offset + q * 4 * 96 * 512,
                    ap=[[512, 102], [96 * 512, 4], [1, 512]],
                )
                sm = smega[i][t % 2]
                engine.dma_start(out=sm[0:102, 4 * q : 4 * q + 4, :], in_=src)

            # per-(img) open psum state
            ones_tiles = [None] * IPC

            def emit_phase1(i, t, w):
                """conv strips for window w."""
                sm = smega[i][t % 2]
                cps = p_conv.tile([128, FD], DT)
                for sp in range(2):
                    nc.tensor.matmul(
                        cps[:, sp * 512 : (sp + 1) * 512],
                        lc_sb[0:102, t * 2 + sp, :],
                        sm[0:102, w, :],
                        start=True,
                        stop=True,
                    )
                return cps

            def emit_lin(i, t, w):
                """lin matmul (x - sum_ch(d)/8 contribution) for window w."""
                sm = smega[i][t % 2]
                j = w % 4
                nc.tensor.matmul(
                    ones_tiles[i][32 * j : 32 * j + 32, :],
                    ln_sb[0:102, t, :],
                    sm[0:102, w, :],
                    start=False,
                    stop=False,
                    skip_group_check=True,
                    tile_position=(0, 32 * j),
                )

            def emit_phase2(i, t, w, cps):
                """elementwise f_res + ones matmul (+ drain on w%4==3)."""
                f8t = ew_f.tile([128, 2, 512], F8)
                is_chain = w in CHAIN_WS
                bias_ap = bia_sb[:, t : t + 1]
                if is_chain:
                    a_t = ch_a.tile([128, FD], BF)
                    nc.scalar.activation(
                        a_t[:, :], cps[:, 0:FD], A.Abs, bias=bias_ap
                    )
                    w_t = ch_w.tile([128, FD], BF)
                    nc.scalar.activation(
                        w_t[:, :], a_t[:, :], A.Square,
                        bias=sqp[:, 0:1], scale=SIGMA ** 0.5,
                    )
                    v_t = ch_v.tile([128, FD], BF)
                    nc.gpsimd.scalar_tensor_tensor(
                        v_t[:, :], cps[:, 0:FD], bias_ap, a_t[:, :],
                        ALU.add, ALU.mult,
                    )
                    nc.gpsimd.scalar_tensor_tensor(
                        f8t[:, :, :], w_t[:, :], SIGMA * Q2_F, v_t[:, :],
                        ALU.add, ALU.mult,
                    )
                else:
                    nc.vector._custom_dve(
                        AD_CUBRB,
                        out=f8t[:, :, :],
                        in0=cps[:, 0:FD],
                        in1=bias_ap,
                        s0=FSC * QC0, s1=FSC * QC1, imm2=FSC * QC2,
                    )
                j = w % 4
                if j == 0:
                    ones_tiles[i] = p_ones.tile([128, 512], DT, name="ones_ps")
                else:
                    emit_lin(i, t, w)
                nc.tensor.matmul(
                    ones_tiles[i][:, :],
                    lo_sb[:, 2 * j + (1 if is_chain else 0), :, :],
                    f8t[:, :, :],
                    start=(j == 0),
                    stop=(j == 3),
                    perf_mode=DR,
                    skip_group_check=True,
                )
                if j == 0:
                    emit_lin(i, t, w)
                if j == 3:
                    c = w // 4
                    drain_bias = bia_sb[:, T + t : T + t + 1]
                    if t < T - 1:
                        nc.scalar.activation(
                            canon[i][:, c, 1:513], ones_tiles[i][:, :],
                            A.Identity, bias=drain_bias,
                        )
                    else:
                        nc.scalar.activation(
                            y_sb[i][:, c, :], ones_tiles[i][:, :],
                            A.Identity, bias=drain_bias,
                        )

            def emit_ywrite(i):
                yi = y_d[i]
                ydst = bass.AP(
                    tensor=yi.tensor,
                    offset=yi.offset,
                    ap=[[512, 128], [512 * 128, NCHUNK], [1, 512]],
                )
                nc.sync.dma_start(out=ydst, in_=y_sb[i][:, :, :])

            # ---- software pipeline ----------------------------------------
            # Per image-step the event list interleaves next-step staging so
            # the step boundary has no serial DMA bubble:
            #   wr(c) right after chunk c's drain (spread across queues);
            #   ld(t+1, q) as soon as the chunks it reads are in V.
            def stream(i):
                ev = []
                for q in range(4):
                    ev.append(("ld", i, 0, q))
                for t in range(T):
                    last = t == T - 1
                    for w in range(NWIN):
                        ev.append(("p1", i, t, w))
                        ev.append(("p2", i, t, w))
                        if w % 4 == 3 and not last:
                            ev.append(("wr", i, t, w // 4))
                        # next-step loads, gated by V writes they depend on:
                        # ld(q) needs chunks q-1, q, q+1 written.
                        if not last:
                            if w == 7:
                                ev.append(("ld", i, t + 1, 0))
                            elif w == 11:
                                ev.append(("ld", i, t + 1, 1))
                    if not last:
                        ev.append(("ld", i, t + 1, 2))
                        ev.append(("ld", i, t + 1, 3))
                    else:
                        ev.append(("yw", i))
                return ev

            LD_ENG = [nc.sync, nc.sync, nc.sync, nc.gpsimd]
            WR_ENG = [nc.sync, nc.scalar, nc.gpsimd]

            def run(ev_list):
                cps_live = {}
                for ev in ev_list:
                    kind = ev[0]
                    if kind == "ld":
                        emit_load(ev[1], ev[2], ev[3], LD_ENG[ev[3]])
                    elif kind == "p1":
                        cps_live[(ev[1], ev[3])] = emit_phase1(ev[1], ev[2], ev[3])
                    elif kind == "p2":
                        emit_phase2(ev[1], ev[2], ev[3], cps_live.pop((ev[1], ev[3])))
                    elif kind == "wr":
                        i, t, c = ev[1], ev[2], ev[3]
                        write_V(i, (t + 1) % 2, canon[i], WR_ENG, c0=c, nch=1)
                    else:
                        emit_ywrite(ev[1])

            sa, sb = stream(0), stream(1)
            STAG = 4 + NWIN  # B trails A by half a step's events
            merged = []
            pa = pb = 0
            while pa < len(sa) or pb < len(sb):
                if pa < len(sa) and (pa < STAG or pa <= pb + STAG):
                    merged.append(sa[pa])
                    pa += 1
                if pb < len(sb) and pa >= STAG:
                    merged.append(sb[pb])
                    pb += 1
            run(merged)

    nc.compile()
    return nc


_NC_CACHE = None


def _get_nc():
    global _NC_CACHE
    if _NC_CACHE is None:
        _NC_CACHE = build_nc()
    return _NC_CACHE


def make_in_maps(x, W, b):
    x = np.asarray(x, np.float32).reshape(N_IMG, H, W_IMG)
    lc, ln, lo, bia = _host_lhst(W, b)
    cstb = np.stack(
        [np.ones(H + 2, np.float32), np.zeros(H + 2, np.float32)]
    ).astype(np.float16)
    return [
        {
            "x": np.ascontiguousarray(x[IPC * c : IPC * (c + 1)]),
            "lc": lc,
            "ln": ln,
            "lo": lo,
            "bia": bia,
            "cstb": cstb,
        }
        for c in range(N_CORES)
    ]


def kernel(x, W, b):
    nc = _get_nc()
    in_maps = make_in_maps(x, W, b)
    res = run_bass_kernel_spmd(nc, in_maps, list(range(N_CORES))).results
    out = np.stack([res[c]["y"] for c in range(N_CORES)])  # [8, 2, 512, 512]
    return out.reshape(N_IMG, 1, H, W_IMG)


if __name__ == "__main__":
    # CoreSim self-test on one core's shard
    from concourse import bass_interp

    rng = np.random.default_rng(0)
    x = rng.standard_normal((IPC, H, W_IMG)).astype(np.float32)
    W = (rng.standard_normal((T, KCH, 1, 3, 3)) * 0.1).astype(np.float32)
    b = (rng.standard_normal((T, KCH)) * 0.1).astype(np.float32)

    def ref_np(x, W, b):
        from scipy.signal import correlate2d

        cur = x.copy()
        for t in range(T):
            d = np.stack(
                [
                    np.stack(
                        [
                            correlate2d(cur[n], W[t, k, 0], mode="same")
                            for k in range(KCH)
                        ]
                    )
                    for n in range(IPC)
                ]
            ) + b[t][None, :, None, None]
            f = np.exp(-np.abs(d) / (1.0 + d * d)) * d
            cur = cur - f.sum(axis=1) / KCH
        return cur

    nc = build_nc()
    lc, ln, lo, bia = _host_lhst(W, b)
    sim = bass_interp.CoreSim(nc)
    sim.tensor("x")[:] = x
    sim.tensor("lc")[:] = lc
    sim.tensor("ln")[:] = ln
    sim.tensor("lo")[:] = lo
    sim.tensor("bia")[:] = bia
    cstf = np.stack([np.ones(H + 2, np.float32), np.zeros(H + 2, np.float32)])
    sim.tensor("cstb")[:] = cstf.astype(np.float16)
    sim.simulate()
    got = sim.tensor("y")
    want = ref_np(x, W, b)
    num = np.linalg.norm((got - want).ravel())
    den = np.linalg.norm(want.ravel())
    print("L2 rel:", num / den)
    print("abs max:", np.abs(got - want).max())


# revision 19
# speedup vs baseline: 2.1299x; 1.0528x over previous
"""Trainium2 Bass kernel for nn_Deep_AD (anisotropic-diffusion CNN).

Math per step t (T=3), on x [N,1,512,512]:
  d  = conv2d_same(x, W[t]) + b[t]          # 1 -> 8 channels, 3x3
  f  = exp(-|d| / (1 + d^2)) * d
  x  = x - sum_ch(f) / 8

Distribution: pure data parallel, 2 images/core on 8 cores (batch shard).

Per-core scheme (G approximated by a cubic in |d|, as baseline):
  x_new = [x - (sum_ch d)/8] - (sum_ch f_res)/8,  f_res = d*(G(|d|)-1)

  * The bracketed LINEAR part (minus the bias term) is computed on the PE:
    per 32-row window one "lin" matmul whose lhsT encodes
    delta - sum_ch(W)/8, reading the same interleaved rhs as the conv. It
    accumulates into the per-chunk psum, so psum ends up holding x_new
    (its -sum(b)/8 constant is added at drain time via a per-partition
    bias AP on the ACT Identity copy).
  * conv: per window 2 strip matmuls [102,128]x[102,512] fp16; banded lhsT
    over the baseline rr-major interleave (rhs partition 3*rr+dx). The
    conv bias is NOT in the matmul; it rides into the elementwise ops.
  * elementwise f_res on d+b: most windows ONE custom DVE op (cubic with
    per-partition bias in1, reads conv psum fp32, writes fp8e4m3 scaled
    x16). Offloaded windows: a=|d+b| [ACT Abs, bias AP],
    w2=sigma*(a+P/2)^2 [ACT Square, scale=sqrt(sigma)], v=(d+b)*a
    [Pool STT, scalar=bias AP], h=(w2+sigma*Q2)*v [Pool STT]; then
    f_res = c2/sigma * h with sign/scale folded into the channel-sum lhsT.
  * channel sum: ONE fp8 DoubleRow matmul per window: the 2 strips are the
    2 k-tiles; selector lhsT routes each strip to its 16 psum rows.
  * drain: ACT Identity copies psum (= x_new) +bias2 to fp16 canon (last
    step: fp32 y_sb).
  * DRAM staging V (baseline layout): 3x row-interleaved shifted image,
    V row 3*R+dx = x_pad row R cols dx..dx+511, fp16. write = 3 DMAs from
    canon (col-shifted reads); loads = 2 half DMAs [102, 8, 512].
"""

import numpy as np
import ml_dtypes

import concourse.bacc as bacc
import concourse.bass as bass
import concourse.tile as tile
from concourse import mybir
from concourse.bass_utils import run_bass_kernel_spmd
from concourse import dve_ops as _dve_ops
from concourse.dve_spec import (
    C0 as _C0,
    C1 as _C1,
    C2 as _C2,
    AluOp as _DAlu,
    Bin as _DBin,
    Spec as _DSpec,
    Src0 as _S0,
    Src1 as _S1,
)


def _register_dve_op(name, spec, perf_en=None):
    """Register a custom DVE op at runtime, probing the uops sha."""
    import re as _re

    for op in _dve_ops.OPS:
        if op.name == name:
            return op
    probe = _dve_ops.DveOp(name, spec, subdim=False, uops_sha={}, perf_en=perf_en or {})
    _dve_ops.OPS.append(probe)
    _dve_ops._SUB_OPCODE_FOR_NAME[name] = _dve_ops._CUSTOM_DVE_ROW_BASE + len(_dve_ops.OPS) - 1
    shas = {}
    for ver in ("v3", "v4"):
        try:
            probe.compile(ver)
        except ValueError as e:
            m = _re.search(r"\(" + ver + r": ([0-9a-f]+) ", str(e))
            if not m:
                raise
            shas[ver] = m.group(1)
    final = _dve_ops.DveOp(
        name, spec, subdim=False, uops_sha=shas, perf_en=perf_en or {}
    )
    _dve_ops.OPS[-1] = final
    _dve_ops.CUSTOM_DVE_SPECS[name] = spec
    return final


# cubic fit of exp(-|d|/(1+d^2)) (weighted by the empirical |d| distribution
# of this problem + a small uniform floor on [0,2.75] to bound tail error).
QC0, QC1, QC2 = -0.91730678, 0.61782336, -0.11857027
FSC = 16.0                       # DVE-path f_res output scale (lo = -1/128)
# factored form: f_res = c2*(a^2 + P*a + Q)*(a*(d+b))
P_F = QC1 / QC2
Q_F = QC0 / QC2
LO_CHAIN = 0.015625              # +2^-6, e4m3-exact; = |c2|/(8*sigma)
SIGMA = abs(QC2) / (8.0 * LO_CHAIN)
Q2_F = Q_F - (P_F / 2.0) ** 2    # w2/sigma = (a + P/2)^2 -> +Q2 completes


def _ref_adcubrb(in0, in1, s0, s1, imm2):
    d = in0.astype(np.float32) + in1.astype(np.float32)
    a = np.abs(d)
    return ((imm2 * a + s1) * a + s0) * a * d


_db = _DBin(_DAlu.ADD, _S0, _S1)
_a = _DBin(_DAlu.ABSOLUTE_VALUE, _db, _db)
_t5 = (((_C2 * _a) + _C1) * _a + _C0) * _a
AD_CUBRB = _register_dve_op(
    "AD_CUBRB",
    _DSpec(body=_t5 * _db, reference=_ref_adcubrb),
)

# problem constants (hardcoded; kernel.py must be self-contained)
T, KCH, H, W_IMG = 3, 8, 512, 512
N_IMG, N_CORES, IPC = 16, 8, 2
WIN, NWIN, NCHUNK = 32, 16, 4      # 32-row windows; 128-row chunks
INR = 34                           # input rows per window (SAME pad)
FD = 1024                          # 2 strips per window flush
NVROW = 3 * (H + 2)                # 1542 interleaved V rows per image
DT = mybir.dt.float32
BF = mybir.dt.float16
F8 = mybir.dt.float8e4
NP8 = ml_dtypes.float8_e4m3
# windows whose elementwise runs on the ACT+Pool chain (per image-step)
CHAIN_WS = frozenset({1, 4, 7, 10, 13})


def _host_lhst(W, b):
    """Host-built stationary tensors.

    lc  [T*2,128,128] f16 : conv lhsT, strip parity sp; row 3*(16sp+g+ky)+kx
                            -> out col 8g+ch (rr-major interleave). No bias.
    ln  [T,128,32]   f16  : lin lhsT; encodes delta - sum_ch(W)/8.
    lo  [8,128,2,128] f8  : ones lhsT variants [2*(w%4)+is_chain]; k-tile kt
                            routes strip 2(w%4)+kt to psum rows 16j+g.
    bia [128, 2*T]   f32  : col t = conv bias b[t,ch] per psum partition;
                            col T+t = -sum(b[t])/8 (drain bias).
    """
    W = np.asarray(W, np.float32)
    b = np.asarray(b, np.float32)
    lc = np.zeros((T, 2, 128, 128), np.float32)
    for t in range(T):
        for sp in range(2):
            for g in range(16):
                for ch in range(KCH):
                    m = 8 * g + ch
                    for ky in range(3):
                        rp = 16 * sp + g + ky
                        for kx in range(3):
                            lc[t, sp, 3 * rp + kx, m] = W[t, ch, 0, ky, kx]
    ln = np.zeros((T, 128, 32), np.float32)
    Wsum = W.sum(axis=1)[:, 0]  # [T,3,3]
    for t in range(T):
        for r in range(32):
            for ky in range(3):
                for kx in range(3):
                    v = -Wsum[t, ky, kx] / KCH
                    if ky == 1 and kx == 1:
                        v += 1.0
                    ln[t, 3 * (r + ky) + kx, r] += v
    lo = np.zeros((4, 2, 128, 2, 128), np.float32)
    for wm in range(4):
        for kt in range(2):
            j = 2 * wm + kt
            for g in range(16):
                for ch in range(KCH):
                    lo[wm, 0, 8 * g + ch, kt, 16 * j + g] = -1.0 / (KCH * FSC)
                    lo[wm, 1, 8 * g + ch, kt, 16 * j + g] = LO_CHAIN
    bia = np.zeros((128, 2 * T), np.float32)
    for t in range(T):
        for g in range(16):
            for ch in range(KCH):
                bia[8 * g + ch, t] = b[t, ch]
        bia[:, T + t] = -b[t].sum() / KCH
    return (
        lc.reshape(T * 2, 128, 128).astype(np.float16),
        ln.astype(np.float16),
        lo.reshape(8, 128, 2, 128).astype(NP8),
        bia,
    )


def build_nc():
    nc = bacc.Bacc(None)
    x_d = nc.declare_dram_parameter("x", [IPC, H, W_IMG], DT, isOutput=False)
    lc_d = nc.declare_dram_parameter("lc", [T * 2, 128, 128], BF, isOutput=False)
    ln_d = nc.declare_dram_parameter("ln", [T, 128, 32], BF, isOutput=False)
    lo_d = nc.declare_dram_parameter("lo", [8, 128, 2, 128], F8, isOutput=False)
    bia_d = nc.declare_dram_parameter("bia", [128, 2 * T], DT, isOutput=False)
    cstb_d = nc.declare_dram_parameter("cstb", [2, H + 2], BF, isOutput=False)
    y_d = nc.declare_dram_parameter("y", [IPC, H, W_IMG], DT, isOutput=True)

    DR = mybir.MatmulPerfMode.DoubleRow
    A = mybir.ActivationFunctionType
    ALU = mybir.AluOpType

    with tile.TileContext(nc) as tc:
        from contextlib import ExitStack

        ctx = ExitStack()
        with ctx:
            singles = ctx.enter_context(tc.tile_pool(name="singles", bufs=1))
            p_conv = ctx.enter_context(
                tc.tile_pool(name="p_conv", bufs=3, space="PSUM")
            )
            p_ones = ctx.enter_context(
                tc.tile_pool(name="p_ones", bufs=2, space="PSUM")
            )
            ew_f = ctx.enter_context(tc.tile_pool(name="ew_f", bufs=10))
            ch_a = ctx.enter_context(tc.tile_pool(name="ch_a", bufs=3))
            ch_w = ctx.enter_context(tc.tile_pool(name="ch_w", bufs=3))
            ch_v = ctx.enter_context(tc.tile_pool(name="ch_v", bufs=3))

            # per-partition scalar const for the chain's Square bias
            sqp = singles.tile([128, 1], DT, name="sqp")
            nc.gpsimd.memset(sqp[:, :], (SIGMA ** 0.5) * P_F / 2.0)

            # stationary operands to SBUF
            lc_sb = singles.tile([128, T * 2, 128], BF)
            nc.sync.dma_start(out=lc_sb, in_=lc_d.rearrange("v k m -> k v m"))
            ln_sb = singles.tile([128, T, 32], BF)
            nc.sync.dma_start(out=ln_sb, in_=ln_d.rearrange("v k m -> k v m"))
            lo_sb = singles.tile([128, 8, 2, 128], F8)
            nc.sync.dma_start(
                out=lo_sb, in_=lo_d.rearrange("v k t m -> k v t m")
            )
            bia_sb = singles.tile([128, 2 * T], DT)
            nc.sync.dma_start(out=bia_sb, in_=bia_d[:, :])

            # canonical x staging [128, chunk, 514] fp16, one per image; only
            # cols 1..512 are written by drains -> zero the padding cols once.
            canon = [
                singles.tile([128, NCHUNK, H + 2], BF, name=f"canon_{i}")
                for i in range(IPC)
            ]
            zc = cstb_d[1:2, 0:1]
            for i in range(IPC):
                zcol = bass.AP(
                    tensor=zc.tensor,
                    offset=zc.offset,
                    ap=[[0, 128], [0, NCHUNK], [1, 1]],
                )
                nc.sync.dma_start(out=canon[i][:, :, 0:1], in_=zcol)
                nc.sync.dma_start(out=canon[i][:, :, 513:514], in_=zcol)

            # final-step output staging fp32
            y_sb = [
                singles.tile([128, NCHUNK, 512], DT, name=f"ysb_{i}")
                for i in range(IPC)
            ]

            # interleaved conv rhs [3*rr+dx, win, col] fp16, double-buffered
            smega = [
                [
                    singles.tile([128, NWIN, 512], BF, name=f"sm_{i}_{p}")
                    for p in range(2)
                ]
                for i in range(IPC)
            ]

            # V: DRAM 3x row-interleaved shifted padded image, per (img, par)
            V = [nc.dram_tensor(f"V_{i}", [2, NVROW, 512], BF) for i in range(IPC)]
            zrow = cstb_d[1:2, 0:1]
            for i in range(IPC):
                for p in range(2):
                    vb = V[i][p]
                    vdst = bass.AP(
                        tensor=vb.tensor,
                        offset=vb.offset,
                        ap=[[(NVROW - 3) * 512, 2], [512, 3], [1, 512]],
                    )
                    vsrc = bass.AP(
                        tensor=zrow.tensor,
                        offset=zrow.offset,
                        ap=[[0, 2], [0, 3], [1, 512]],
                    )
                    nc.sync.dma_start(out=vdst, in_=vsrc)

            def write_V(i, par, src_tile, engine, c):
                """canon chunk c -> V[i][par]: ONE DMA writes all 3 dx-shifted
                interleaved rows (V rows 3R+dx adjacent; src free dims
                (dx, col) overlap at stride 1)."""
                vb = V[i][par]
                vdst = bass.AP(
                    tensor=vb.tensor,
                    offset=vb.offset + 3 * (128 * c + 1) * 512,
                    ap=[[3 * 512, 128], [512, 3], [1, 512]],
                )
                st = src_tile[:, c, 0:512]
                vsrc = bass.AP(
                    tensor=st.tensor,
                    offset=st.offset,
                    ap=[[514 * NCHUNK, 128], [1, 3], [1, 512]],
                )
                engine.dma_start(out=vdst, in_=vsrc)

            # load input into canon (cast via gpsimd), build V[i][0]
            for i in range(IPC):
                xi = x_d[i]
                xsrc = bass.AP(
                    tensor=xi.tensor,
                    offset=xi.offset,
                    ap=[[512, 128], [512 * 128, NCHUNK], [1, 512]],
                )
                nc.gpsimd.dma_start(out=canon[i][:, :, 1:513], in_=xsrc)
                for c in range(NCHUNK):
                    write_V(i, 0, canon[i], nc.sync, c)

            # ---- per-step events ------------------------------------------
            def emit_load(i, t, q, engine):
                """quarter-load: windows 4q..4q+3 of step t into smega."""
                vb = V[i][t % 2]
                src = bass.AP(
                    tensor=vb.tensor,
                    offset=vb.offset + q * 4 * 96 * 512,
                    ap=[[512, 102], [96 * 512, 4], [1, 512]],
                )
                sm = smega[i][t % 2]
                engine.dma_start(out=sm[0:102, 4 * q : 4 * q + 4, :], in_=src)

            # per-(img) open psum state
            ones_tiles = [None] * IPC

            def emit_phase1(i, t, w):
                """conv strips for window w."""
                sm = smega[i][t % 2]
                cps = p_conv.tile([128, FD], DT)
                for sp in range(2):
                    nc.tensor.matmul(
                        cps[:, sp * 512 : (sp + 1) * 512],
                        lc_sb[0:102, t * 2 + sp, :],
                        sm[0:102, w, :],
                        start=True,
                        stop=True,
                    )
                return cps

            def emit_lin(i, t, w):
                """lin matmul (x - sum_ch(d)/8 contribution) for window w."""
                sm = smega[i][t % 2]
                j = w % 4
                nc.tensor.matmul(
                    ones_tiles[i][32 * j : 32 * j + 32, :],
                    ln_sb[0:102, t, :],
                    sm[0:102, w, :],
                    start=False,
                    stop=False,
                    skip_group_check=True,
                    tile_position=(0, 32 * j),
                )

            def emit_phase2(i, t, w, cps):
                """elementwise f_res + ones matmul (+ drain on w%4==3)."""
                f8t = ew_f.tile([128, 2, 512], F8)
                is_chain = w in CHAIN_WS
                bias_ap = bia_sb[:, t : t + 1]
                if is_chain:
                    a_t = ch_a.tile([128, FD], BF)
                    nc.scalar.activation(
                        a_t[:, :], cps[:, 0:FD], A.Abs, bias=bias_ap
                    )
                    w_t = ch_w.tile([128, FD], BF)
                    nc.scalar.activation(
                        w_t[:, :], a_t[:, :], A.Square,
                        bias=sqp[:, 0:1], scale=SIGMA ** 0.5,
                    )
                    v_t = ch_v.tile([128, FD], BF)
                    nc.gpsimd.scalar_tensor_tensor(
                        v_t[:, :], cps[:, 0:FD], bias_ap, a_t[:, :],
                        ALU.add, ALU.mult,
                    )
                    nc.gpsimd.scalar_tensor_tensor(
                        f8t[:, :, :], w_t[:, :], SIGMA * Q2_F, v_t[:, :],
                        ALU.add, ALU.mult,
                    )
                else:
                    nc.vector._custom_dve(
                        AD_CUBRB,
                        out=f8t[:, :, :],
                        in0=cps[:, 0:FD],
                        in1=bias_ap,
                        s0=FSC * QC0, s1=FSC * QC1, imm2=FSC * QC2,
                    )
                j = w % 4
                if j == 0:
                    ones_tiles[i] = p_ones.tile([128, 512], DT, name="ones_ps")
                else:
                    emit_lin(i, t, w)
                nc.tensor.matmul(
                    ones_tiles[i][:, :],
                    lo_sb[:, 2 * j + (1 if is_chain else 0), :, :],
                    f8t[:, :, :],
                    start=(j == 0),
                    stop=(j == 3),
                    perf_mode=DR,
                    skip_group_check=True,
                )
                if j == 0:
                    emit_lin(i, t, w)
                if j == 3:
                    c = w // 4
                    drain_bias = bia_sb[:, T + t : T + t + 1]
                    if t < T - 1:
                        nc.scalar.activation(
                            canon[i][:, c, 1:513], ones_tiles[i][:, :],
                            A.Identity, bias=drain_bias,
                        )
                    else:
                        nc.scalar.activation(
                            y_sb[i][:, c, :], ones_tiles[i][:, :],
                            A.Identity, bias=drain_bias,
                        )

            def emit_ywrite(i):
                yi = y_d[i]
                ydst = bass.AP(
                    tensor=yi.tensor,
                    offset=yi.offset,
                    ap=[[512, 128], [512 * 128, NCHUNK], [1, 512]],
                )
                nc.sync.dma_start(out=ydst, in_=y_sb[i][:, :, :])

            # ---- software pipeline ----------------------------------------
            # Per image-step the event list interleaves next-step staging so
            # the step boundary has no serial DMA bubble:
            #   wr(c) right after chunk c's drain (spread across queues);
            #   ld(t+1, q) as soon as the chunks it reads are in V.
            def stream(i):
                ev = []
                for q in range(4):
                    ev.append(("ld", i, 0, q))
                for t in range(T):
                    last = t == T - 1
                    for w in range(NWIN):
                        ev.append(("p1", i, t, w))
                        ev.append(("p2", i, t, w))
                        if w % 4 == 3 and not last:
                            ev.append(("wr", i, t, w // 4))
                        # next-step loads, gated by V writes they depend on:
                        # ld(q) needs chunks q-1, q, q+1 written.
                        if not last:
                            if w == 7:
                                ev.append(("ld", i, t + 1, 0))
                            elif w == 11:
                                ev.append(("ld", i, t + 1, 1))
                    if not last:
                        ev.append(("ld", i, t + 1, 2))
                        ev.append(("ld", i, t + 1, 3))
                    else:
                        ev.append(("yw", i))
                return ev

            LD_ENG = [nc.sync, nc.sync, nc.sync, nc.sync]

            def run(ev_list):
                cps_live = {}
                for ev in ev_list:
                    kind = ev[0]
                    if kind == "ld":
                        emit_load(ev[1], ev[2], ev[3], LD_ENG[ev[3]])
                    elif kind == "p1":
                        cps_live[(ev[1], ev[3])] = emit_phase1(ev[1], ev[2], ev[3])
                    elif kind == "p2":
                        emit_phase2(ev[1], ev[2], ev[3], cps_live.pop((ev[1], ev[3])))
                    elif kind == "wr":
                        i, t, c = ev[1], ev[2], ev[3]
                        write_V(i, (t + 1) % 2, canon[i], nc.sync, c)
                    else:
                        emit_ywrite(ev[1])

            sa, sb = stream(0), stream(1)
            STAG = 4 + NWIN  # B trails A by half a step's events
            merged = []
            pa = pb = 0
            while pa < len(sa) or pb < len(sb):
                if pa < len(sa) and (pa < STAG or pa <= pb + STAG):
                    merged.append(sa[pa])
                    pa += 1
                if pb < len(sb) and pa >= STAG:
                    merged.append(sb[pb])
                    pb += 1
            run(merged)

    nc.compile()
    return nc


_NC_CACHE = None


def _get_nc():
    global _NC_CACHE
    if _NC_CACHE is None:
        _NC_CACHE = build_nc()
    return _NC_CACHE


def make_in_maps(x, W, b):
    x = np.asarray(x, np.float32).reshape(N_IMG, H, W_IMG)
    lc, ln, lo, bia = _host_lhst(W, b)
    cstb = np.stack(
        [np.ones(H + 2, np.float32), np.zeros(H + 2, np.float32)]
    ).astype(np.float16)
    return [
        {
            "x": np.ascontiguousarray(x[IPC * c : IPC * (c + 1)]),
            "lc": lc,
            "ln": ln,
            "lo": lo,
            "bia": bia,
            "cstb": cstb,
        }
        for c in range(N_CORES)
    ]


def kernel(x, W, b):
    nc = _get_nc()
    in_maps = make_in_maps(x, W, b)
    res = run_bass_kernel_spmd(nc, in_maps, list(range(N_CORES))).results
    out = np.stack([res[c]["y"] for c in range(N_CORES)])  # [8, 2, 512, 512]
    return out.reshape(N_IMG, 1, H, W_IMG)


if __name__ == "__main__":
    # CoreSim self-test on one core's shard
    from concourse import bass_interp

    rng = np.random.default_rng(0)
    x = rng.standard_normal((IPC, H, W_IMG)).astype(np.float32)
    W = (rng.standard_normal((T, KCH, 1, 3, 3)) * 0.1).astype(np.float32)
    b = (rng.standard_normal((T, KCH)) * 0.1).astype(np.float32)

    def ref_np(x, W, b):
        from scipy.signal import correlate2d

        cur = x.copy()
        for t in range(T):
            d = np.stack(
                [
                    np.stack(
                        [
                            correlate2d(cur[n], W[t, k, 0], mode="same")
                            for k in range(KCH)
                        ]
                    )
                    for n in range(IPC)
                ]
            ) + b[t][None, :, None, None]
            f = np.exp(-np.abs(d) / (1.0 + d * d)) * d
            cur = cur - f.sum(axis=1) / KCH
        return cur

    nc = build_nc()
    lc, ln, lo, bia = _host_lhst(W, b)
    sim = bass_interp.CoreSim(nc)
    sim.tensor("x")[:] = x
    sim.tensor("lc")[:] = lc
    sim.tensor("ln")[:] = ln
    sim.tensor("lo")[:] = lo
    sim.tensor("bia")[:] = bia
    cstf = np.stack([np.ones(H + 2, np.float32), np.zeros(H + 2, np.float32)])
    sim.tensor("cstb")[:] = cstf.astype(np.float16)
    sim.simulate()
    got = sim.tensor("y")
    want = ref_np(x, W, b)
    num = np.linalg.norm((got - want).ravel())
    den = np.linalg.norm(want.ravel())
    print("L2 rel:", num / den)
    print("abs max:", np.abs(got - want).max())
